# revision 22
# baseline (speedup 1.0000x reference)
"""Trainium2 Bass kernel for nn_AttentionModel (sparse_attention).

Reference computation:
    x = emb_table[tokens]                  # [B,S,D]
    scores = x @ x^T per batch             # [B,S,S]
    out = softmax(scores) @ x              # [B,S,D]
    logits = out[:, 0, :] @ cls_w.T + cls_b

Only row 0 of the attention output is used, and that row only ever meets
cls_w, so per batch element the whole model reduces to

    q = x[0]
    s_t = <x_t, q>                 (2048 dot products of length 512)
    e = exp(s);  Z = sum(e)
    logits_c = sum_t e_t * y[tok_t, c] / Z + b_c,   y = emb_table @ cls_w^T

Device strategy (data-parallel over batch, 8 cores x 4 sequences):

  * The table is uploaded as fp8(emb*32) [32000, 512] (512B rows).
    dma_gather(transpose=True) fetches each sequence's 2048 rows directly in
    d-major layout: XT[p, cu, t, eps] = fp8 x_t[256*cu + 2*p + eps]. Token 0
    doubles as the query column. Gathers are chunked at 512 indices across 4
    SWDGE queues (larger single instructions corrupt the descriptor ring;
    the queues also parallelize Q7 descriptor generation on hardware).
  * Scores run on the PE as stationary-weight matmuls (contraction dim d on
    partitions, 128-token output columns), psum-accumulated over (cu, eps).
    exp + per-partition softmax sums happen in one scalar-engine activation
    reading psum (scale folds away the fp8 *32 scaling).
  * y = emb @ cls_w^T is host-precomputed weight prep; the per-token y pairs
    (32KB/core) are host-laid-out token-major alongside the indices, and the
    softmax numerator sum_t e_t y_t is 16 accumulating [128,1]x[128,2]
    matmuls. No DVE bulk work anywhere.
"""

import numpy as np

import bass_rust

import concourse.bass as bass
import concourse.mybir as mybir
import concourse.tile as tile
from concourse.bass_utils import run_bass_kernel_spmd


def _split_multiwaits(nc: bass.Bass) -> None:
    """Workaround for the walrus build in this container, which rejects
    instructions carrying more than one sync-wait command ("Too many sync
    wait commands" / "ISA wrong length" in CoreV3GenImpl setupSyncWait).

    Moves each instruction's sync waits onto dedicated single-wait NOPs
    inserted right before it on the same engine stream (bass_nofuse so
    walrus's nop-fusion can't merge them back)."""
    counter = 0
    fn = nc.m.functions[0]
    for bb in fn.blocks:
        insts = bb.instructions
        new_list = []
        changed = False
        for inst in insts:
            si = inst.sync_info
            waits = list(si.on_wait) if si is not None else []
            if waits:
                for w in waits:
                    counter += 1
                    new_list.append(
                        mybir.InstNoOp(
                            name=f"waitnop-{counter}",
                            engine=inst.engine,
                            ins=[],
                            outs=[],
                            bass_nofuse=True,
                            sync_info=bass_rust.SyncInfo(on_wait=[w], on_update=[]),
                        )
                    )
                inst.sync_info = bass_rust.SyncInfo(
                    on_wait=[], on_update=list(si.on_update)
                )
                changed = True
            new_list.append(inst)
        if changed:
            bb.instructions = new_list


def _bacc_postpasses(nc: bass.Bass) -> None:
    """GPSIMD extended instructions (InstDMAGatherAnt) need their Q7 library
    load inserted and ISA payload bytes generated — Bacc does this in
    compile(); plain Bass does not."""
    from concourse.library_config import all_libraries, standard

    mask: dict = {}
    for lib in all_libraries:
        for it in lib.instructions:
            mask[it] = mask.get(it, 0) | (1 << lib.index)
    bass_rust.insert_library_loads(nc, mask, len(all_libraries), standard.index)
    mybir.codegen_inst_isa_subclasses(nc)


B, S, D, V, C = 32, 2048, 512, 32000, 2
N_CORES = 8
BPC = B // N_CORES          # sequences per core
NCH = 4                     # gather chunks per sequence
CH = S // NCH               # 512 indices per gather
JT = S // 128               # 16 token tiles per sequence
EMB_SCALE = 32.0            # emb is quantized as fp8(emb*32); scores carry 32^2

F32 = mybir.dt.float32
BF16 = mybir.dt.bfloat16
FP8 = mybir.dt.float8e4
I16 = mybir.dt.int16

_CACHE: dict = {}


def _build_nc() -> bass.Bass:
    nc = bass.Bass(dynamic_dma_scratch_size=2**17, num_swdge_queues=4)
    emb_d = nc.dram_tensor("emb8", [V, D], FP8, kind="ExternalInput")
    idx_d = nc.dram_tensor("idx", [128, BPC * (S // 16)], I16, kind="ExternalInput")
    cb_d = nc.dram_tensor("cls_b", [1, C], F32, kind="ExternalInput")
    yt_d = nc.dram_tensor("yt", [128, BPC * JT * C], BF16, kind="ExternalInput")
    out_d = nc.dram_tensor("out", [BPC, C], F32, kind="ExternalOutput")

    mult = mybir.AluOpType.mult
    add = mybir.AluOpType.add
    EXP = mybir.ActivationFunctionType.Exp

    with tile.TileContext(nc) as tc:
        with (
            tc.tile_pool(name="const", bufs=1) as constp,
            tc.tile_pool(name="xp", bufs=BPC) as xp,
            tc.tile_pool(name="sp", bufs=BPC) as sp,
            tc.tile_pool(name="ps", bufs=2, space="PSUM") as pp,
        ):
            idx = constp.tile([128, BPC * (S // 16)], I16)
            nc.sync.dma_start(idx[:], idx_d[:, :])
            yt = constp.tile([128, BPC, JT, C], BF16)
            nc.sync.dma_start(yt[:, :, :, :], yt_d[:, :])
            cb = constp.tile([1, C], F32)
            nc.sync.dma_start(cb[:], cb_d[:, :])
            ones128 = constp.tile([128, 1], F32)
            nc.vector.memset(ones128[:], 1.0)

            for b in range(BPC):
                # --- transpose-gather this sequence's rows ---
                # xt[p, g, cu, t', eps] = fp8 x_{CH*g+t'}[256*cu + 2*p + eps]
                xt = xp.tile([128, NCH, 2, CH, 2], FP8, tag="xt")
                for g in range(NCH):
                    gout = (
                        xt[:, g, :, :, :]
                        .rearrange("p cu t e -> p (cu t e)")
                        .rearrange("p (a b) -> p a b", a=4)
                    )
                    nc.gpsimd.dma_gather(
                        out_ap=gout,
                        in_ap=emb_d[:, :],
                        idxs_ap=idx[:, b * (S // 16) + g * (CH // 16):
                                    b * (S // 16) + (g + 1) * (CH // 16)],
                        num_idxs=CH,
                        num_idxs_reg=CH,
                        elem_size=D,
                        transpose=True,
                        queue_num=g % 4,
                    )

                # --- scores: s[t] = <x_t, q>, q = token-0 column ---
                spm = pp.tile([128, JT], F32, tag="spm")
                for j in range(JT):
                    g, jj = divmod(j, CH // 128)
                    first = True
                    for cu in range(2):
                        for eps in range(2):
                            nc.tensor.matmul(
                                spm[:, j:j + 1],
                                xt[:, g, cu, 128 * jj:128 * (jj + 1), eps],
                                xt[:, 0, cu, 0:1, eps],
                                start=first,
                                stop=(cu == 1 and eps == 1),
                            )
                            first = False

                # --- softmax pieces ---
                e = sp.tile([128, JT], BF16, tag="e")
                zcol = sp.tile([128, 1], F32, tag="zcol")
                nc.scalar.activation(
                    e[:], spm[:], EXP,
                    scale=1.0 / (EMB_SCALE * EMB_SCALE),
                    accum_out=zcol[:],
                )
                # --- numerator and Z ---
                npm_t = pp.tile([1, C], F32, tag="npm")
                npm = npm_t[:]
                for j in range(JT):
                    nc.tensor.matmul(
                        npm, e[:, j:j + 1], yt[:, b, j, :],
                        start=(j == 0), stop=(j == JT - 1),
                    )
                zpm_t = pp.tile([1, 1], F32, tag="zpm")
                zpm = zpm_t[:]
                nc.tensor.matmul(zpm, zcol[:], ones128[:], start=True, stop=True)

                rz = sp.tile([1, 1], F32, tag="rz")
                nc.vector.reciprocal(rz[:], zpm)

                ob = sp.tile([1, C], F32, tag="ob")
                nc.vector.scalar_tensor_tensor(
                    ob[:], npm, rz[:], cb[:], op0=mult, op1=add
                )
                nc.sync.dma_start(out_d[b:b + 1, :], ob[:])

    nc.finalize()
    _bacc_postpasses(nc)
    _split_multiwaits(nc)
    return nc


def get_nc() -> bass.Bass:
    if "nc" not in _CACHE:
        _CACHE["nc"] = _build_nc()
    return _CACHE["nc"]


def _prep_tables(emb_table: np.ndarray, cls_w: np.ndarray):
    import ml_dtypes

    emb = np.asarray(emb_table, dtype=np.float32)
    y = (emb @ np.asarray(cls_w, dtype=np.float32).T).astype(ml_dtypes.bfloat16)
    emb8 = (emb * EMB_SCALE).astype(ml_dtypes.float8_e4m3fn)
    return emb8, y


def _build_yt(tokens: np.ndarray, y: np.ndarray) -> np.ndarray:
    """Per-core [128, BPC*JT*C] bf16: yt[p, b, j, c] = y[tok[b, 128*j + p], c]
    (token-major, matching the gather's score-column layout)."""
    out = np.empty((128, BPC, JT, C), y.dtype)
    for b in range(BPC):
        out[:, b] = y[tokens[b]].reshape(JT, 128, C).transpose(1, 0, 2)
    return out.reshape(128, BPC * JT * C)


def _build_idx(tokens: np.ndarray) -> np.ndarray:
    """Per-core [128, BPC*128] int16; token t of sequence b sits at
    [16*g + t%16, b*128 + t//16] for every 16-partition group g (the SWDGE
    TX core reads group 1; CoreSim reads group 0)."""
    toks = tokens.astype(np.int16)          # [BPC, S], values < 32000
    slot = np.empty((16, BPC * (S // 16)), np.int16)
    for b in range(BPC):
        slot[:, b * (S // 16):(b + 1) * (S // 16)] = toks[b].reshape(S // 16, 16).T
    return np.tile(slot, (8, 1))


def make_in_maps(tokens, emb_table, cls_w, cls_b):
    tokens = np.asarray(tokens)
    emb8, y = _prep_tables(emb_table, cls_w)
    cb = np.ascontiguousarray(np.asarray(cls_b, dtype=np.float32)).reshape(1, C)
    in_maps = []
    for core in range(N_CORES):
        ct = tokens[core * BPC:(core + 1) * BPC]
        in_maps.append(
            {
                "emb8": emb8,
                "idx": _build_idx(ct),
                "yt": _build_yt(ct, y),
                "cls_b": cb,
            }
        )
    return in_maps


def kernel(tokens, emb_table, cls_w, cls_b) -> np.ndarray:
    nc = get_nc()
    in_maps = make_in_maps(tokens, emb_table, cls_w, cls_b)
    res = run_bass_kernel_spmd(nc, in_maps, core_ids=list(range(N_CORES)))
    outs = [res.results[c]["out"] for c in range(N_CORES)]
    return np.concatenate(outs, axis=0).astype(np.float32)
